# revision 17
# baseline (speedup 1.0000x reference)
"""Trainium2 Bass kernel for the GODEFunc graph-ODE message-passing module.

Math (per batch b):
    xa   = sum_k conv_w[k] * (adj[k] @ x[b]) + conv_b
    W    = (w * clip(d,0,1)) @ w.T
    out  = tanh(0.5*sigmoid(alpha) * xa - 2*x[b] + x[b] @ W + x0[b]*sigmoid(beta))

Sharding: rows (nodes) split across 8 cores; each core computes its
1024-row slice of the output for all batches.  No collectives needed.

Structure (v4):
  - The K axis is folded on the host (the 1x1 conv over K is linear):
    adjc = cw0*adj0 + cw1*adj1.  The alpha gate 0.5*sigmoid(alpha[row])
    is ALSO folded into adj rows on the host, so the device-side scale
    is the literal constant 1/S.
  - adj is pre-scaled by S and cast to fp8 e4m3 on the host; x is cast
    to fp8.  adj traffic per core drops 64MB -> 8.4MB.
  - Main matmuls run DoubleRow fp8 (one instruction contracts TWO
    128-deep chunks) with x stationary and the adj stream moving; the
    PSUM output is the TRANSPOSED result [bf, rows], un-transposed on
    the host.  The PE is the critical path (throttle-limited), so the
    schedule aims to start it early and keep it dense.
  - The whole xw = x@(W-2I) + x0*sigmoid(beta) + bias path is
    precomputed on the host and uploaded transposed (1MB/core).
  - Epilogue per psum region: acc = psum/S + xwx0T, tanh -> bf16 out.
  - DMA: the adj stream head is tapered (2,2,4 chunks) and rides the
    sync HWDGE queue together with the first xs group, so the first
    matmul can issue ~8us earlier than a pure-SWDGE stream; the SWDGE
    queue carries the 8-chunk body groups, the scalar queue carries
    xwx0 + the remaining xs groups, and the two output halves leave on
    different queues.
"""

import sys

for _p in ("/opt/trn_rl_repo",):
    if _p not in sys.path:
        sys.path.insert(0, _p)

from contextlib import ExitStack

import numpy as np
import ml_dtypes

import concourse.bass as bass
import concourse.mybir as mybir
import concourse.tile as tile
from concourse import bacc
from concourse.bass_utils import run_bass_kernel_spmd

dt = mybir.dt
AF = mybir.ActivationFunctionType
ALU = mybir.AluOpType
PM = mybir.MatmulPerfMode

B, N, F, K = 4, 8192, 64, 2
N_CORES = 8
P = 128
S = 16384.0  # adj fp8 pre-scale; epilogue multiplies psum by 1/S
FP8 = getattr(ml_dtypes, "float8_e4m3", ml_dtypes.float8_e4m3fn)

NS = N // N_CORES  # 1024 rows per core
MC = N // P        # 64 contraction chunks
NG = 8             # xs DMA groups
GC = MC // NG      # 8 chunks per xs group
BF = B * F         # 256 stacked batch-feature columns
NH = BF // P       # 2 bf halves (psum partition groups)
NR = NS // BF      # 4 row blocks of 256 per psum region row

# adj stream groups (start_chunk, n_chunks): the head is tapered small
# so the PE's first weights+moving operands land as early as possible.
AGROUPS = [(0, 2), (2, 2), (4, 4), (8, 8), (16, 8), (24, 8), (32, 8),
           (40, 8), (48, 8), (56, 8)]
N_SYNC_AG = 3      # first 3 adj groups ride the sync HWDGE queue
CHUNK_ELEMS = P * NS  # elements per adj chunk in the flat HBM buffer


def build_kernel():
    """Build the per-core Bass module.  All cores run the same program on
    their own row shard."""
    nc = bacc.Bacc(None, target_bir_lowering=False, debug=False)

    # Flat group-blocked adj buffer: for each group (c0, n) in AGROUPS,
    # the range [c0*CHUNK_ELEMS, (c0+n)*CHUNK_ELEMS) holds the block
    # [p, c, r] = S * 0.5*sigmoid(alpha[row0+r]) * adjc[row0+r,
    # (c0+c)*128+p]  (fully contiguous per group).
    adjq = nc.dram_tensor("adjq", [MC * CHUNK_ELEMS], dt.float8e4,
                          kind="ExternalInput")
    # xs[g, p, gc, b*F+f] = x[b, (g*GC+gc)*128+p, f] (fp8, shared by all
    # cores; group-major so each group DMA is a contiguous 256KB read)
    xs = nc.dram_tensor("xs", [NG, P, GC, BF], dt.float8e4,
                        kind="ExternalInput")
    # xwx0T[h, p_bf, r] = (x@(W-2I) + x0*sigmoid(beta) +
    #                      0.5*sigmoid(alpha)*conv_b)[b, row0+r, f]
    # with b*F+f = h*128+p_bf  (transposed to match the psum layout)
    xwx0T = nc.dram_tensor("xwx0T", [NH, P, NS], dt.float32,
                           kind="ExternalInput")
    # transposed output: y_tT[h, p_bf, r] (bf16; host upcasts)
    y_tT = nc.dram_tensor("y_tT", [NH, P, NS], dt.bfloat16,
                          kind="ExternalOutput")

    with tile.TileContext(nc) as tc, ExitStack() as ctx:
        const = ctx.enter_context(tc.tile_pool(name="const", bufs=1))
        adjp = ctx.enter_context(tc.tile_pool(name="adjp", bufs=4))
        adjh = ctx.enter_context(tc.tile_pool(name="adjh", bufs=1))
        outp = ctx.enter_context(tc.tile_pool(name="outp", bufs=2))
        keep = ctx.enter_context(tc.tile_pool(name="keep", bufs=1))
        psy = ctx.enter_context(tc.tile_pool(name="psy", bufs=1, space="PSUM"))

        a_tiles = {}

        def emit_adj_dma(gi, eng):
            c0, n = AGROUPS[gi]
            head = n != GC
            pool = adjh if head else adjp
            tag = f"adj{gi}" if head else "adj"
            a_t = pool.tile([P, n, NS], dt.float8e4, tag=tag, name=f"a{gi}")
            eng.dma_start(
                out=a_t[:],
                in_=adjq[c0 * CHUNK_ELEMS : (c0 + n) * CHUNK_ELEMS],
            )
            a_tiles[gi] = a_t

        xs_sb = {}

        def emit_xs_dma(g, eng):
            t = const.tile([P, GC, BF], dt.float8e4, tag=f"xs{g}",
                           name=f"xs_sb{g}")
            eng.dma_start(out=t[:], in_=xs[g])
            xs_sb[g] = t

        # start order: the PE needs xs g0 + the small adj head groups
        # first; they all ride the sync HWDGE queue (lower first-packet
        # latency than SWDGE).  The SWDGE queue streams the 8-chunk adj
        # body; the later xs groups alternate between the sync and
        # scalar queues ahead of when the PE reaches them, and xwx0
        # (needed only by the epilogue) trails on the scalar queue.
        emit_xs_dma(0, nc.sync)
        for gi in range(N_SYNC_AG):
            emit_adj_dma(gi, nc.sync)
        for gi in range(N_SYNC_AG, len(AGROUPS)):
            emit_adj_dma(gi, nc.gpsimd)
        for g in range(1, NG):
            emit_xs_dma(g, nc.scalar if g % 2 == 0 else nc.sync)

        xwx0_sb = []
        for h in range(NH):
            t = const.tile([P, NS], dt.float32, tag=f"xwx0{h}",
                           name=f"xwx0_sb{h}")
            nc.scalar.dma_start(out=t[:], in_=xwx0T[h])
            xwx0_sb.append(t)

        # 8 psum regions of [128, 256] f32: region (h, rb) packs two per
        # bank
        psum_t = [
            psy.tile([P, 2 * BF], dt.float32, tag=f"y{i}", name=f"psum_t{i}")
            for i in range(NH * NR // 2)
        ]

        def region(h, rb):
            i = h * NR + rb
            return psum_t[i // 2][:, (i % 2) * BF : (i % 2 + 1) * BF]

        out_bfT = [
            keep.tile([P, NS], dt.bfloat16, tag=f"out_bf{h}", name=f"out_bfT{h}")
            for h in range(NH)
        ]

        N_PAIRS = MC // 2

        def emit_pair(gi, j, h_order=(0, 1)):
            """One chunk pair: per bf half, 4 row-block DoubleRow matmuls
            with x stationary and the adj stream moving."""
            c0, _n = AGROUPS[gi]
            a_t = a_tiles[gi]
            cg = c0 + 2 * j              # global chunk index (even)
            gp = cg // 2                 # global pair index
            for h in h_order:
                w_ap = xs_sb[cg // GC][:, (cg % GC) : (cg % GC) + 2,
                                       h * P : (h + 1) * P]
                for rb in range(NR):
                    nc.tensor.matmul(
                        region(h, rb),
                        w_ap,
                        a_t[:, 2 * j : 2 * j + 2, rb * BF : (rb + 1) * BF],
                        start=(gp == 0),
                        stop=(gp == N_PAIRS - 1),
                        perf_mode=PM.DoubleRow,
                        skip_group_check=True,
                    )

        def emit_epilogue(h, out_eng):
            # out = tanh(psum/S + xwx0T) for one bf half (4 regions)
            for rb in range(NR):
                acc = outp.tile([P, BF], dt.float32, tag="eacc")
                nc.vector.scalar_tensor_tensor(
                    acc[:], region(h, rb), 1.0 / S,
                    xwx0_sb[h][:, rb * BF : (rb + 1) * BF],
                    ALU.mult, ALU.add,
                )
                nc.scalar.activation(
                    out_bfT[h][:, rb * BF : (rb + 1) * BF], acc[:], AF.Tanh
                )
            out_eng.dma_start(out=y_tT[h], in_=out_bfT[h][:])

        for gi in range(len(AGROUPS) - 1):
            for j in range(AGROUPS[gi][1] // 2):
                emit_pair(gi, j)
        # last group: half-major so half 0's epilogue + output DMA
        # overlap half 1's matmuls; the two halves leave on different
        # queues.
        gi = len(AGROUPS) - 1
        for h in range(NH):
            for j in range(AGROUPS[gi][1] // 2):
                emit_pair(gi, j, h_order=(h,))
            emit_epilogue(h, nc.sync if h == 0 else nc.scalar)

    nc.finalize()
    return nc


_NC_CACHE = {}


def _get_nc(key=0):
    if key not in _NC_CACHE:
        _NC_CACHE[key] = build_kernel()
    return _NC_CACHE[key]


def _sigmoid(v):
    return 1.0 / (1.0 + np.exp(-v))


def make_in_maps(x, x0, adj, alpha, beta, w, d, conv_w, conv_b,
                 n_cores=N_CORES):
    """Fold + re-lay the full inputs into per-core shards."""
    f32 = np.float32
    x = np.asarray(x, f32)
    x0 = np.asarray(x0, f32)
    adj = np.asarray(adj, f32)
    alpha = np.asarray(alpha, f32)
    beta = np.asarray(beta, f32)
    w = np.asarray(w, f32)
    d = np.asarray(d, f32)
    conv_w = np.asarray(conv_w, f32)
    conv_b = np.asarray(conv_b, f32)

    # fold the K axis (1x1 conv is linear) and the alpha gate into adj
    adjc = conv_w[0] * adj[0]
    for k in range(1, adj.shape[0]):
        adjc += conv_w[k] * adj[k]
    gate = 0.5 * _sigmoid(alpha)  # [N] per output row
    adjq_T = np.ascontiguousarray(
        (adjc * (gate * f32(S))[:, None]).astype(FP8).T
    )  # [m, row]

    # xs[g, p, gc, b*F+f] = x[b, (g*GC+gc)*128+p, f] (shared by all cores)
    xs_full = np.ascontiguousarray(
        x.reshape(B, NG, GC, P, F).transpose(1, 3, 2, 0, 4)
        .reshape(NG, P, GC, BF)
    ).astype(FP8)

    # host-side xw path: z = x@(W-2I) + x0*sigmoid(beta) + gate*conv_b
    wp = (w * np.clip(d, 0.0, 1.0)[None, :]) @ w.T - 2.0 * np.eye(F, dtype=f32)
    z = x @ wp + x0 * _sigmoid(beta)[None, :, None] \
        + (gate * conv_b[0])[None, :, None]
    z = z.astype(f32)  # [B, N, F]

    in_maps = []
    for c in range(n_cores):
        rows = slice(c * NS, (c + 1) * NS)
        # per-group blocks [p, ch, r], flattened in AGROUPS order
        core_cols = adjq_T[:, rows].reshape(MC, P, NS)
        adjq_c = np.concatenate(
            [
                np.ascontiguousarray(
                    core_cols[c0 : c0 + n].transpose(1, 0, 2)
                ).reshape(-1)
                for c0, n in AGROUPS
            ]
        )
        # z[:, rows] [B, NS, F] -> [bf, r] -> [NH, P, NS]
        zT_c = np.ascontiguousarray(
            z[:, rows].transpose(0, 2, 1).reshape(NH, P, NS), dtype=f32
        )
        in_maps.append({"adjq": adjq_c, "xs": xs_full, "xwx0T": zT_c})
    return in_maps


def unshard(results):
    # y_tT[h, p_bf, r] -> y[b, c*NS + r, f] with b*F+f = h*128+p_bf
    parts = [
        np.asarray(results[c]["y_tT"]).reshape(BF, NS).T.reshape(NS, B, F)
        .transpose(1, 0, 2)
        for c in range(N_CORES)
    ]
    return np.concatenate(parts, axis=1).astype(np.float32)


def kernel(x, x0, adj, alpha, beta, w, d, conv_w, conv_b):
    nc = _get_nc()
    in_maps = make_in_maps(x, x0, adj, alpha, beta, w, d, conv_w, conv_b)
    res = run_bass_kernel_spmd(nc, in_maps, core_ids=list(range(N_CORES)))
    return unshard(res.results)


# revision 19
# speedup vs baseline: 1.0985x; 1.0985x over previous
"""Trainium2 Bass kernel for the GODEFunc graph-ODE message-passing module.

Math (per batch b):
    xa   = sum_k conv_w[k] * (adj[k] @ x[b]) + conv_b
    W    = (w * clip(d,0,1)) @ w.T
    out  = tanh(0.5*sigmoid(alpha) * xa - 2*x[b] + x[b] @ W + x0[b]*sigmoid(beta))

Sharding: rows (nodes) split across 8 cores; each core computes its
1024-row slice of the output for all batches.  No collectives needed.

Structure (v4):
  - The K axis is folded on the host (the 1x1 conv over K is linear):
    adjc = cw0*adj0 + cw1*adj1.  The alpha gate 0.5*sigmoid(alpha[row])
    is ALSO folded into adj rows on the host, so the device-side scale
    is the literal constant 1/S.
  - adj is pre-scaled by S and cast to fp8 e4m3 on the host; x is cast
    to fp8.  adj traffic per core drops 64MB -> 8.4MB.
  - Main matmuls run DoubleRow fp8 (one instruction contracts TWO
    128-deep chunks) with x stationary and the adj stream moving; the
    PSUM output is the TRANSPOSED result [bf, rows], un-transposed on
    the host.  The PE is the critical path (throttle-limited), so the
    schedule aims to start it early and keep it dense.
  - The whole xw = x@(W-2I) + x0*sigmoid(beta) + bias path is
    precomputed on the host and uploaded transposed (1MB/core).
  - Epilogue per psum region: acc = psum/S + xwx0T, tanh -> bf16 out.
  - DMA: the adj stream head is tapered (2,2,4 chunks) and rides the
    sync HWDGE queue together with the first xs group, so the first
    matmul can issue ~8us earlier than a pure-SWDGE stream; the SWDGE
    queue carries the 8-chunk body groups, the scalar queue carries
    xwx0 + the remaining xs groups, and the two output halves leave on
    different queues.
"""

import sys

for _p in ("/opt/trn_rl_repo",):
    if _p not in sys.path:
        sys.path.insert(0, _p)

from contextlib import ExitStack

import numpy as np
import ml_dtypes

import concourse.bass as bass
import concourse.mybir as mybir
import concourse.tile as tile
from concourse import bacc
from concourse.bass_utils import run_bass_kernel_spmd

dt = mybir.dt
AF = mybir.ActivationFunctionType
ALU = mybir.AluOpType
PM = mybir.MatmulPerfMode

B, N, F, K = 4, 8192, 64, 2
N_CORES = 8
P = 128
S = 16384.0  # adj fp8 pre-scale; epilogue multiplies psum by 1/S
FP8 = getattr(ml_dtypes, "float8_e4m3", ml_dtypes.float8_e4m3fn)

NS = N // N_CORES  # 1024 rows per core
MC = N // P        # 64 contraction chunks
NG = 8             # xs DMA groups
GC = MC // NG      # 8 chunks per xs group
BF = B * F         # 256 stacked batch-feature columns
NH = BF // P       # 2 bf halves (psum partition groups)
NR = NS // BF      # 4 row blocks of 256 per psum region row

# adj stream groups (start_chunk, n_chunks): the head is tapered small
# so the PE's first weights+moving operands land as early as possible.
AGROUPS = [(0, 2), (2, 2), (4, 4), (8, 8), (16, 8), (24, 8), (32, 8),
           (40, 8), (48, 8), (56, 8)]
N_SYNC_AG = 3      # first 3 adj groups ride the sync HWDGE queue
CHUNK_ELEMS = P * NS  # elements per adj chunk in the flat HBM buffer


def build_kernel():
    """Build the per-core Bass module.  All cores run the same program on
    their own row shard."""
    nc = bacc.Bacc(None, target_bir_lowering=False, debug=False)

    # Flat group-blocked adj buffer: for each group (c0, n) in AGROUPS,
    # the range [c0*CHUNK_ELEMS, (c0+n)*CHUNK_ELEMS) holds the block
    # [p, c, r] = S * 0.5*sigmoid(alpha[row0+r]) * adjc[row0+r,
    # (c0+c)*128+p]  (fully contiguous per group).
    adjq = nc.dram_tensor("adjq", [MC * CHUNK_ELEMS], dt.float8e4,
                          kind="ExternalInput")
    # xs[g, p, gc, b*F+f] = x[b, (g*GC+gc)*128+p, f] (fp8, shared by all
    # cores; group-major so each group DMA is a contiguous 256KB read)
    xs = nc.dram_tensor("xs", [NG, P, GC, BF], dt.float8e4,
                        kind="ExternalInput")
    # xwx0T[h, p_bf, r] = (x@(W-2I) + x0*sigmoid(beta) +
    #                      0.5*sigmoid(alpha)*conv_b)[b, row0+r, f]
    # with b*F+f = h*128+p_bf  (transposed to match the psum layout)
    xwx0T = nc.dram_tensor("xwx0T", [NH, P, NS], dt.float32,
                           kind="ExternalInput")
    # transposed output: y_tT[h, p_bf, r] (bf16; host upcasts)
    y_tT = nc.dram_tensor("y_tT", [NH, P, NS], dt.bfloat16,
                          kind="ExternalOutput")

    with tile.TileContext(nc) as tc, ExitStack() as ctx:
        const = ctx.enter_context(tc.tile_pool(name="const", bufs=1))
        adjp = ctx.enter_context(tc.tile_pool(name="adjp", bufs=5))
        adjh = ctx.enter_context(tc.tile_pool(name="adjh", bufs=1))
        outp = ctx.enter_context(tc.tile_pool(name="outp", bufs=2))
        keep = ctx.enter_context(tc.tile_pool(name="keep", bufs=1))
        psy = ctx.enter_context(tc.tile_pool(name="psy", bufs=1, space="PSUM"))

        a_tiles = {}

        def emit_adj_dma(gi, eng):
            c0, n = AGROUPS[gi]
            head = n != GC
            pool = adjh if head else adjp
            tag = f"adj{gi}" if head else "adj"
            a_t = pool.tile([P, n, NS], dt.float8e4, tag=tag, name=f"a{gi}")
            eng.dma_start(
                out=a_t[:],
                in_=adjq[c0 * CHUNK_ELEMS : (c0 + n) * CHUNK_ELEMS],
            )
            a_tiles[gi] = a_t

        xs_sb = {}

        def emit_xs_dma(g, eng):
            t = const.tile([P, GC, BF], dt.float8e4, tag=f"xs{g}",
                           name=f"xs_sb{g}")
            eng.dma_start(out=t[:], in_=xs[g])
            xs_sb[g] = t

        # start order: the whole critical stream (xs g0, the tapered adj
        # head, then the adj body) rides the SWDGE queue, which sustains
        # ~5x the HWDGE rate on these per-partition run sizes.  The
        # HWDGE queues only carry lookahead xs groups (needed ~8+ us
        # after the PE starts) and the epilogue-only xwx0.
        emit_xs_dma(0, nc.gpsimd)
        for gi in range(len(AGROUPS)):
            emit_adj_dma(gi, nc.gpsimd)
        for g in range(1, NG):
            emit_xs_dma(g, nc.scalar if g % 2 == 0 else nc.sync)

        xwx0_sb = []
        for h in range(NH):
            t = const.tile([P, NS], dt.float32, tag=f"xwx0{h}",
                           name=f"xwx0_sb{h}")
            nc.scalar.dma_start(out=t[:], in_=xwx0T[h])
            xwx0_sb.append(t)

        # 8 psum regions of [128, 256] f32: region (h, rb) packs two per
        # bank
        psum_t = [
            psy.tile([P, 2 * BF], dt.float32, tag=f"y{i}", name=f"psum_t{i}")
            for i in range(NH * NR // 2)
        ]

        def region(h, rb):
            i = h * NR + rb
            return psum_t[i // 2][:, (i % 2) * BF : (i % 2 + 1) * BF]

        out_bfT = [
            keep.tile([P, NS], dt.bfloat16, tag=f"out_bf{h}", name=f"out_bfT{h}")
            for h in range(NH)
        ]

        N_PAIRS = MC // 2

        def emit_pair(gi, j, h_order=(0, 1)):
            """One chunk pair: per bf half, 4 row-block DoubleRow matmuls
            with x stationary and the adj stream moving."""
            c0, _n = AGROUPS[gi]
            a_t = a_tiles[gi]
            cg = c0 + 2 * j              # global chunk index (even)
            gp = cg // 2                 # global pair index
            for h in h_order:
                w_ap = xs_sb[cg // GC][:, (cg % GC) : (cg % GC) + 2,
                                       h * P : (h + 1) * P]
                for rb in range(NR):
                    nc.tensor.matmul(
                        region(h, rb),
                        w_ap,
                        a_t[:, 2 * j : 2 * j + 2, rb * BF : (rb + 1) * BF],
                        start=(gp == 0),
                        stop=(gp == N_PAIRS - 1),
                        perf_mode=PM.DoubleRow,
                        skip_group_check=True,
                    )

        def emit_epilogue(h, out_eng):
            # out = tanh(psum/S + xwx0T) for one bf half (4 regions)
            for rb in range(NR):
                acc = outp.tile([P, BF], dt.float32, tag="eacc")
                nc.vector.scalar_tensor_tensor(
                    acc[:], region(h, rb), 1.0 / S,
                    xwx0_sb[h][:, rb * BF : (rb + 1) * BF],
                    ALU.mult, ALU.add,
                )
                nc.scalar.activation(
                    out_bfT[h][:, rb * BF : (rb + 1) * BF], acc[:], AF.Tanh
                )
            out_eng.dma_start(out=y_tT[h], in_=out_bfT[h][:])

        for gi in range(len(AGROUPS) - 1):
            for j in range(AGROUPS[gi][1] // 2):
                emit_pair(gi, j)
        # last group: half-major so half 0's epilogue + output DMA
        # overlap half 1's matmuls; the two halves leave on different
        # queues.
        gi = len(AGROUPS) - 1
        for h in range(NH):
            for j in range(AGROUPS[gi][1] // 2):
                emit_pair(gi, j, h_order=(h,))
            emit_epilogue(h, nc.sync if h == 0 else nc.scalar)

    nc.finalize()
    return nc


_NC_CACHE = {}


def _get_nc(key=0):
    if key not in _NC_CACHE:
        _NC_CACHE[key] = build_kernel()
    return _NC_CACHE[key]


def _sigmoid(v):
    return 1.0 / (1.0 + np.exp(-v))


def make_in_maps(x, x0, adj, alpha, beta, w, d, conv_w, conv_b,
                 n_cores=N_CORES):
    """Fold + re-lay the full inputs into per-core shards."""
    f32 = np.float32
    x = np.asarray(x, f32)
    x0 = np.asarray(x0, f32)
    adj = np.asarray(adj, f32)
    alpha = np.asarray(alpha, f32)
    beta = np.asarray(beta, f32)
    w = np.asarray(w, f32)
    d = np.asarray(d, f32)
    conv_w = np.asarray(conv_w, f32)
    conv_b = np.asarray(conv_b, f32)

    # fold the K axis (1x1 conv is linear) and the alpha gate into adj
    adjc = conv_w[0] * adj[0]
    for k in range(1, adj.shape[0]):
        adjc += conv_w[k] * adj[k]
    gate = 0.5 * _sigmoid(alpha)  # [N] per output row
    adjq_T = np.ascontiguousarray(
        (adjc * (gate * f32(S))[:, None]).astype(FP8).T
    )  # [m, row]

    # xs[g, p, gc, b*F+f] = x[b, (g*GC+gc)*128+p, f] (shared by all cores)
    xs_full = np.ascontiguousarray(
        x.reshape(B, NG, GC, P, F).transpose(1, 3, 2, 0, 4)
        .reshape(NG, P, GC, BF)
    ).astype(FP8)

    # host-side xw path: z = x@(W-2I) + x0*sigmoid(beta) + gate*conv_b
    wp = (w * np.clip(d, 0.0, 1.0)[None, :]) @ w.T - 2.0 * np.eye(F, dtype=f32)
    z = x @ wp + x0 * _sigmoid(beta)[None, :, None] \
        + (gate * conv_b[0])[None, :, None]
    z = z.astype(f32)  # [B, N, F]

    in_maps = []
    for c in range(n_cores):
        rows = slice(c * NS, (c + 1) * NS)
        # per-group blocks [p, ch, r], flattened in AGROUPS order
        core_cols = adjq_T[:, rows].reshape(MC, P, NS)
        adjq_c = np.concatenate(
            [
                np.ascontiguousarray(
                    core_cols[c0 : c0 + n].transpose(1, 0, 2)
                ).reshape(-1)
                for c0, n in AGROUPS
            ]
        )
        # z[:, rows] [B, NS, F] -> [bf, r] -> [NH, P, NS]
        zT_c = np.ascontiguousarray(
            z[:, rows].transpose(0, 2, 1).reshape(NH, P, NS), dtype=f32
        )
        in_maps.append({"adjq": adjq_c, "xs": xs_full, "xwx0T": zT_c})
    return in_maps


def unshard(results):
    # y_tT[h, p_bf, r] -> y[b, c*NS + r, f] with b*F+f = h*128+p_bf
    parts = [
        np.asarray(results[c]["y_tT"]).reshape(BF, NS).T.reshape(NS, B, F)
        .transpose(1, 0, 2)
        for c in range(N_CORES)
    ]
    return np.concatenate(parts, axis=1).astype(np.float32)


def kernel(x, x0, adj, alpha, beta, w, d, conv_w, conv_b):
    nc = _get_nc()
    in_maps = make_in_maps(x, x0, adj, alpha, beta, w, d, conv_w, conv_b)
    res = run_bass_kernel_spmd(nc, in_maps, core_ids=list(range(N_CORES)))
    return unshard(res.results)


# revision 20
# speedup vs baseline: 1.2527x; 1.1404x over previous
"""Trainium2 Bass kernel for the GODEFunc graph-ODE message-passing module.

Math (per batch b):
    xa   = sum_k conv_w[k] * (adj[k] @ x[b]) + conv_b
    W    = (w * clip(d,0,1)) @ w.T
    out  = tanh(0.5*sigmoid(alpha) * xa - 2*x[b] + x[b] @ W + x0[b]*sigmoid(beta))

Sharding: rows (nodes) split across 8 cores; each core computes its
1024-row slice of the output for all batches.  No collectives needed.

Structure (v7):
  - Host folding: adjc = cw0*adj0 + cw1*adj1 (the 1x1 conv over K is
    linear), with the alpha gate 0.5*sigmoid(alpha[row]) folded into
    adj rows, pre-scaled by S and cast to fp8 e4m3 (device-side scale
    is the literal 1/S).  x is cast to fp8.  The whole
    xw = x@(W-2I) + x0*sigmoid(beta) + bias path is precomputed on the
    host and uploaded transposed (1MB/core).
  - One fused input stream: each chunk group's adj block AND its xs
    slice live in one contiguous HBM block ([P, n*(NS+BF)] fp8,
    ~10KB-per-partition runs), so a single SWDGE DMA per group feeds
    both matmul operands at the queue's best packet size (~240+ GB/s).
    Group sizes taper at the head (2,2,4) so the PE starts early, and
    at the tail (4,2,2) so the last matmuls + epilogue trail the
    stream end by ~2us.
  - Main matmuls run DoubleRow fp8 (one instruction contracts TWO
    128-deep chunks) with x stationary and the adj stream moving; psum
    holds the TRANSPOSED result [bf, rows], un-transposed on the host.
  - Epilogue per psum region: acc = psum/S + xwx0T, tanh -> bf16,
    output halves leave on the SWDGE queue (the HWDGE queues are ~5x
    slower at these run sizes and only carry the epilogue-only xwx0).
"""

import sys

for _p in ("/opt/trn_rl_repo",):
    if _p not in sys.path:
        sys.path.insert(0, _p)

from contextlib import ExitStack

import numpy as np
import ml_dtypes

import concourse.bass as bass
import concourse.mybir as mybir
import concourse.tile as tile
from concourse import bacc
from concourse.bass_utils import run_bass_kernel_spmd

dt = mybir.dt
AF = mybir.ActivationFunctionType
ALU = mybir.AluOpType
PM = mybir.MatmulPerfMode

B, N, F, K = 4, 8192, 64, 2
N_CORES = 8
P = 128
S = 16384.0  # adj fp8 pre-scale; epilogue multiplies psum by 1/S
FP8 = getattr(ml_dtypes, "float8_e4m3", ml_dtypes.float8_e4m3fn)

NS = N // N_CORES  # 1024 rows per core
MC = N // P        # 64 contraction chunks
BF = B * F         # 256 stacked batch-feature columns
NH = BF // P       # 2 bf halves (psum partition groups)
NR = NS // BF      # 4 row blocks of 256 per psum region row
N_PAIRS = MC // 2  # 32 chunk pairs

# fused stream groups (start_chunk, n_chunks): tapered at both ends
AGROUPS = [(0, 2), (2, 2), (4, 4), (8, 8), (16, 8), (24, 8), (32, 8),
           (40, 8), (48, 8), (56, 4), (60, 2), (62, 2)]
GW = NS + BF       # fused per-chunk width per partition (adj + xs)


def build_kernel():
    """Build the per-core Bass module.  All cores run the same program on
    their own row shard."""
    nc = bacc.Bacc(None, target_bir_lowering=False, debug=False)

    # Flat group-blocked fused buffer: for each group (c0, n) in
    # AGROUPS, the range [c0*P*GW, (c0+n)*P*GW) holds the block
    # [p, n*NS adj | n*BF xs]:
    #   adj part [c, r]: S * 0.5*sigmoid(alpha[row0+r]) * adjc[row0+r,
    #                    (c0+c)*128+p]
    #   xs part  [c, b*F+f]: x[b, (c0+c)*128+p, f]
    fused = nc.dram_tensor("fused", [MC * P * GW], dt.float8e4,
                           kind="ExternalInput")
    # xwx0T[h, p_bf, r] = (x@(W-2I) + x0*sigmoid(beta) +
    #                      0.5*sigmoid(alpha)*conv_b)[b, row0+r, f]
    # with b*F+f = h*128+p_bf  (transposed to match the psum layout)
    xwx0T = nc.dram_tensor("xwx0T", [NH, P, NS], dt.float32,
                           kind="ExternalInput")
    # transposed output: y_tT[h, p_bf, r] (bf16; host upcasts)
    y_tT = nc.dram_tensor("y_tT", [NH, P, NS], dt.bfloat16,
                          kind="ExternalOutput")

    with tile.TileContext(nc) as tc, ExitStack() as ctx:
        const = ctx.enter_context(tc.tile_pool(name="const", bufs=1))
        adjp = ctx.enter_context(tc.tile_pool(name="adjp", bufs=5))
        adjh = ctx.enter_context(tc.tile_pool(name="adjh", bufs=1))
        outp = ctx.enter_context(tc.tile_pool(name="outp", bufs=2))
        keep = ctx.enter_context(tc.tile_pool(name="keep", bufs=1))
        psy = ctx.enter_context(tc.tile_pool(name="psy", bufs=1, space="PSUM"))

        g_tiles = {}

        def emit_group_dma(gi):
            c0, n = AGROUPS[gi]
            body = n == 8
            pool = adjp if body else adjh
            tag = "adj" if body else f"adj{gi}"
            t = pool.tile([P, n * GW], dt.float8e4, tag=tag, name=f"a{gi}")
            nc.gpsimd.dma_start(
                out=t[:], in_=fused[c0 * P * GW : (c0 + n) * P * GW]
            )
            g_tiles[gi] = t

        for gi in range(len(AGROUPS)):
            emit_group_dma(gi)

        xwx0_sb = []
        for h in range(NH):
            t = const.tile([P, NS], dt.float32, tag=f"xwx0{h}",
                           name=f"xwx0_sb{h}")
            (nc.sync if h == 0 else nc.scalar).dma_start(
                out=t[:], in_=xwx0T[h]
            )
            xwx0_sb.append(t)

        # 8 psum regions of [128, 256] f32: region (h, rb) packs two per
        # bank
        psum_t = [
            psy.tile([P, 2 * BF], dt.float32, tag=f"y{i}", name=f"psum_t{i}")
            for i in range(NH * NR // 2)
        ]

        def region(h, rb):
            i = h * NR + rb
            return psum_t[i // 2][:, (i % 2) * BF : (i % 2 + 1) * BF]

        out_bfT = [
            keep.tile([P, NS], dt.bfloat16, tag=f"out_bf{h}", name=f"out_bfT{h}")
            for h in range(NH)
        ]

        def emit_pair(gi, j, h_order=(0, 1)):
            """One chunk pair: per bf half, 4 row-block DoubleRow matmuls
            with x stationary and the adj stream moving."""
            c0, n = AGROUPS[gi]
            t = g_tiles[gi]
            adj_v = t[:, : n * NS].rearrange("p (c r) -> p c r", c=n)
            xs_v = t[:, n * NS :].rearrange("p (c bf) -> p c bf", c=n)
            gp = (c0 + 2 * j) // 2       # global pair index
            for h in h_order:
                w_ap = xs_v[:, 2 * j : 2 * j + 2, h * P : (h + 1) * P]
                for rb in range(NR):
                    nc.tensor.matmul(
                        region(h, rb),
                        w_ap,
                        adj_v[:, 2 * j : 2 * j + 2, rb * BF : (rb + 1) * BF],
                        start=(gp == 0),
                        stop=(gp == N_PAIRS - 1),
                        perf_mode=PM.DoubleRow,
                        skip_group_check=True,
                    )

        def emit_epilogue(h):
            # out = tanh(psum/S + xwx0T) for one bf half (4 regions);
            # the output rides the fast SWDGE queue.
            for rb in range(NR):
                acc = outp.tile([P, BF], dt.float32, tag="eacc")
                nc.vector.scalar_tensor_tensor(
                    acc[:], region(h, rb), 1.0 / S,
                    xwx0_sb[h][:, rb * BF : (rb + 1) * BF],
                    ALU.mult, ALU.add,
                )
                nc.scalar.activation(
                    out_bfT[h][:, rb * BF : (rb + 1) * BF], acc[:], AF.Tanh
                )
            nc.gpsimd.dma_start(out=y_tT[h], in_=out_bfT[h][:])

        for gi in range(len(AGROUPS) - 1):
            for j in range(AGROUPS[gi][1] // 2):
                emit_pair(gi, j)
        # last group (one pair): half-major so half 0's epilogue
        # overlaps half 1's matmuls
        gi = len(AGROUPS) - 1
        for h in range(NH):
            emit_pair(gi, 0, h_order=(h,))
            emit_epilogue(h)

    nc.finalize()
    return nc


_NC_CACHE = {}


def _get_nc(key=0):
    if key not in _NC_CACHE:
        _NC_CACHE[key] = build_kernel()
    return _NC_CACHE[key]


def _sigmoid(v):
    return 1.0 / (1.0 + np.exp(-v))


def make_in_maps(x, x0, adj, alpha, beta, w, d, conv_w, conv_b,
                 n_cores=N_CORES):
    """Fold + re-lay the full inputs into per-core shards."""
    f32 = np.float32
    x = np.asarray(x, f32)
    x0 = np.asarray(x0, f32)
    adj = np.asarray(adj, f32)
    alpha = np.asarray(alpha, f32)
    beta = np.asarray(beta, f32)
    w = np.asarray(w, f32)
    d = np.asarray(d, f32)
    conv_w = np.asarray(conv_w, f32)
    conv_b = np.asarray(conv_b, f32)

    # fold the K axis (1x1 conv is linear) and the alpha gate into adj
    adjc = conv_w[0] * adj[0]
    for k in range(1, adj.shape[0]):
        adjc += conv_w[k] * adj[k]
    gate = 0.5 * _sigmoid(alpha)  # [N] per output row
    adjq_T = np.ascontiguousarray(
        (adjc * (gate * f32(S))[:, None]).astype(FP8).T
    )  # [m, row]

    # xs_c[c, p, b*F+f] = x[b, c*128+p, f] (shared by all cores)
    xs_c = np.ascontiguousarray(
        x.reshape(B, MC, P, F).transpose(1, 2, 0, 3).reshape(MC, P, BF)
    ).astype(FP8)

    # host-side xw path: z = x@(W-2I) + x0*sigmoid(beta) + gate*conv_b
    wp = (w * np.clip(d, 0.0, 1.0)[None, :]) @ w.T - 2.0 * np.eye(F, dtype=f32)
    z = x @ wp + x0 * _sigmoid(beta)[None, :, None] \
        + (gate * conv_b[0])[None, :, None]
    z = z.astype(f32)  # [B, N, F]

    in_maps = []
    for c in range(n_cores):
        rows = slice(c * NS, (c + 1) * NS)
        core_cols = adjq_T[:, rows].reshape(MC, P, NS)
        # fused per-group blocks [p, n*NS adj | n*BF xs], flat
        blocks = []
        for c0, n in AGROUPS:
            adj_blk = core_cols[c0 : c0 + n].transpose(1, 0, 2).reshape(P, -1)
            xs_blk = xs_c[c0 : c0 + n].transpose(1, 0, 2).reshape(P, -1)
            blocks.append(
                np.ascontiguousarray(
                    np.concatenate([adj_blk, xs_blk], axis=1)
                ).reshape(-1)
            )
        fused_c = np.concatenate(blocks)
        # z[:, rows] [B, NS, F] -> [bf, r] -> [NH, P, NS]
        zT_c = np.ascontiguousarray(
            z[:, rows].transpose(0, 2, 1).reshape(NH, P, NS), dtype=f32
        )
        in_maps.append({"fused": fused_c, "xwx0T": zT_c})
    return in_maps


def unshard(results):
    # y_tT[h, p_bf, r] -> y[b, c*NS + r, f] with b*F+f = h*128+p_bf
    parts = [
        np.asarray(results[c]["y_tT"]).reshape(BF, NS).T.reshape(NS, B, F)
        .transpose(1, 0, 2)
        for c in range(N_CORES)
    ]
    return np.concatenate(parts, axis=1).astype(np.float32)


def kernel(x, x0, adj, alpha, beta, w, d, conv_w, conv_b):
    nc = _get_nc()
    in_maps = make_in_maps(x, x0, adj, alpha, beta, w, d, conv_w, conv_b)
    res = run_bass_kernel_spmd(nc, in_maps, core_ids=list(range(N_CORES)))
    return unshard(res.results)
